# revision 22
# baseline (speedup 1.0000x reference)
"""Single-core dual-layer Trainium2 kernel for the 2-layer ReLU-RNN.

  B=64, T=512, I=256, H=512, O=1
  layer l: h_t = relu(x_t @ W_ih^T + b_ih + b_hh + h_{t-1} @ W_hh^T)
  out = sigmoid(h1 @ W_fc^T + b_fc)

v3 design (microbench-driven): the RNN scan is LATENCY-bound — each step
costs ~max(PE_work, P + L_sync) where the relu->matmul semaphore round
trip L (~1.5-2us) far exceeds the 16-block matmul burst P (~0.3us with one
16-col matmul per 128x128 weight block). That latency shadow absorbs all
other work for free, so each core runs EVERYTHING for its 16-sample block:

  - L0 chain: chunk j in iteration j (CH=16 steps per chunk)
  - L1 chain: chunk j-2 in iteration j (single-parity PSUM, refilled by
    trailing sub-thunks right behind the consuming relu)
  - L0 input projection (xt GEMM) for chunk j+1, interleaved as PE filler
  - L1 input projection (h0 GEMM) for chunk j-1, sub-split by 4-step
    ranges so each sub-thunk lands after the relu that frees its PSUM cols
  - FC + sigmoid for chunk j-3

4 cores cover B=64; no collectives, no pipeline-fill iterations beyond 3.
Relu split: ACT handles L0, DVE handles L1 — each chain's next step waits
on a single semaphore, keeping the PE wait queue shallow.

PSUM budget: L0 ps [128,1024]f32 = 2 banks x2 parity + L1 ps x1 + fc x2
(1 bank each) = 8 banks exactly.
"""

import numpy as np
import ml_dtypes

import concourse.bass as bass
import concourse.mybir as mybir
from concourse.tile import TileContext
from concourse.bass_utils import run_bass_kernel_spmd
from concourse.alu_op_type import AluOpType

F16 = np.float16
NCORES = 4
B, T, I, H, O = 64, 512, 256, 512, 1
NW = 16            # samples per core (one merged moving group)
CH = 16            # steps per chunk
KH = H // 128      # 4
KX = I // 128      # 2
CW = CH * NW       # cols per m-chunk in a psum/h tile = 256
HC = KH * CW       # h-tile cols = 1024
LAG1 = 2           # L1 chain lag in chunks
LAGF = 3           # fc lag in chunks

_ctr = [0]


def _split_multi_waits(nc):
    """Walrus in this container rejects >1 sync-wait per instruction."""
    n_split = 0
    for f in nc.m.functions:
        for bb in f.blocks:
            out = []
            changed = False
            for inst in bb.instructions:
                si = inst.sync_info
                waits = list(si.on_wait) if si is not None and si.on_wait else []
                if len(waits) > 1:
                    changed = True
                    n_split += 1
                    for w in waits[:-1]:
                        _ctr[0] += 1
                        nop = mybir.InstNoOp(
                            name=f"waitnop-{_ctr[0]}", ins=[], outs=[]
                        )
                        nop.engine = inst.engine
                        nop.sync_info = mybir.SyncInfo(on_wait=[w], on_update=[])
                        out.append(nop)
                    inst.sync_info = mybir.SyncInfo(
                        on_wait=[waits[-1]],
                        on_update=list(si.on_update) if si.on_update else [],
                    )
                out.append(inst)
            if changed:
                bb.instructions = out
    return n_split


def build_nc(n_steps=T, debug_dumps=False, no_fc=False, dbg_l1_mode=0):
    nch = n_steps // CH
    niter = nch + LAGF
    nc = bass.Bass("TRN2", num_devices=NCORES)
    f32 = mybir.dt.float32
    bf = mybir.dt.float16

    xt_d = nc.dram_tensor("xt", [128, KX * n_steps * NW], bf,
                          kind="ExternalInput")
    wg0_d = nc.dram_tensor("wg0", [128, KX * H], bf, kind="ExternalInput")
    wg1_d = nc.dram_tensor("wg1", [128, KH * H], bf, kind="ExternalInput")
    wh0_d = nc.dram_tensor("wh0", [128, KH * H], bf, kind="ExternalInput")
    wh1_d = nc.dram_tensor("wh1", [128, KH * H], bf, kind="ExternalInput")
    b0_d = nc.dram_tensor("b0", [KH, 128], bf, kind="ExternalInput")
    b1_d = nc.dram_tensor("b1", [KH, 128], bf, kind="ExternalInput")
    wfc_d = nc.dram_tensor("wfc", [128, KH * 8], bf, kind="ExternalInput")
    ind_d = nc.dram_tensor("ind", [KH, HC], bf, kind="ExternalInput")
    bfc_d = nc.dram_tensor("bfc", [1, 1], f32, kind="ExternalInput")
    y_d = nc.dram_tensor("y", [1, nch * CW], f32, kind="ExternalOutput")
    if debug_dumps:
        dh_d = nc.dram_tensor("dh", [128, niter * 2 * HC], bf,
                              kind="ExternalOutput")

    with TileContext(nc) as tc:
        with (
            tc.tile_pool(name="wts", bufs=1) as p_w,
            tc.tile_pool(name="h0", bufs=3) as p_h0,
            tc.tile_pool(name="h1", bufs=3) as p_h1,
            tc.tile_pool(name="y", bufs=2) as p_y,
            tc.tile_pool(name="ps", bufs=1, space="PSUM") as p_ps,
        ):
            # ---- load inputs ----
            xt_sb = p_w.tile([128, KX * n_steps * NW], bf, tag="xt")
            nc.sync.dma_start(xt_sb[:], xt_d[:])
            wg0_sb = p_w.tile([128, KX * H], bf, tag="wg0")
            nc.sync.dma_start(wg0_sb[:], wg0_d[:])
            wg1_sb = p_w.tile([128, KH * H], bf, tag="wg1")
            nc.sync.dma_start(wg1_sb[:], wg1_d[:])
            wh0_sb = p_w.tile([128, KH * H], bf, tag="wh0")
            nc.sync.dma_start(wh0_sb[:], wh0_d[:])
            wh1_sb = p_w.tile([128, KH * H], bf, tag="wh1")
            nc.sync.dma_start(wh1_sb[:], wh1_d[:])
            b0_sb = p_w.tile([KH, 128], bf, tag="b0")
            nc.sync.dma_start(b0_sb[:], b0_d[:])
            b1_sb = p_w.tile([KH, 128], bf, tag="b1")
            nc.sync.dma_start(b1_sb[:], b1_d[:])
            wfc_sb = p_w.tile([128, KH * 8], bf, tag="wfc")
            nc.sync.dma_start(wfc_sb[:], wfc_d[:])
            bfc_sb = p_w.tile([1, 1], f32, tag="bfc")
            nc.sync.dma_start(bfc_sb[:], bfc_d[:])
            ind_sb = p_w.tile([KH, HC], bf, tag="ind")
            nc.sync.dma_start(ind_sb[:], ind_d[:])
            h0_zero = p_w.tile([128, HC], bf, tag="h0z")
            nc.vector.memset(h0_zero[:], 0.0)
            h1_zero = p_w.tile([128, HC], bf, tag="h1z")
            nc.vector.memset(h1_zero[:], 0.0)

            ps0_tiles = {}  # chunk -> L0 psum tile
            h0_t = {-1: h0_zero}
            h1_t = {-1: h1_zero}

            def mslice(ps, m, r=None):
                if r is None:
                    return ps[:, m * CW:(m + 1) * CW]
                return ps[:, m * CW + r * NW:m * CW + (r + 1) * NW]

            # ---- L0 input GEMM: prefill ps0 for chunk c ----
            def l0_thunks(c):
                ps = p_ps.tile([128, HC], f32, tag=f"ps0{c % 2}",
                               name=f"ps0_{c}")
                ps0_tiles[c] = ps
                thunks = [
                    lambda hb=hb, ps=ps: nc.tensor.matmul(
                        ps[:, hb * 2 * CW:(hb + 1) * 2 * CW], b0_sb[:],
                        ind_sb[:, hb * 2 * CW:(hb + 1) * 2 * CW],
                        start=True, stop=False)
                    for hb in range(2)
                ]
                for k in range(KX):
                    for m in range(KH):
                        w = wg0_sb[:, k * H + m * 128:k * H + (m + 1) * 128]
                        off = (k * n_steps + c * CH) * NW
                        thunks.append(
                            lambda m=m, w=w, off=off, k=k, ps=ps:
                            nc.tensor.matmul(
                                mslice(ps, m), w, xt_sb[:, off:off + CW],
                                start=False, stop=(k == KX - 1),
                            ))
                return thunks

            # ps1: single psum region reused for every L1 chunk
            ps1 = p_ps.tile([128, HC], f32, tag="ps1", name="ps1")

            # ---- L1 refill for chunk c, steps 4q..4q+3. ps1 is single-
            # buffered and shared with the in-flight chain, so NO matmul here
            # may use start=True: start clears the ENTIRE psum bank
            # (first_mm semantics), wiping slices the chain still needs.
            # Instead a DVE memset (emitted right after the relu that frees
            # these cols) zeroes them, and bias + proj accumulate onto zero
            # with start=False. ----
            ps1_m = ps1[:].rearrange("p (m x) -> p m x", m=KH)
            HW2 = CH // 2 * NW   # cols per half-chunk within an m-block

            def l1_memset(half):
                nc.vector.memset(
                    ps1_m[:, :, half * HW2:(half + 1) * HW2], 0.0)

            def l1_subthunk(c, half):
                thunks = []
                h0 = h0_t[c]
                for m in range(KH):
                    cs = m * CW + half * HW2
                    thunks.append(
                        lambda m=m, cs=cs: nc.tensor.matmul(
                            ps1[:, cs:cs + HW2], b1_sb[:],
                            ind_sb[:, cs:cs + HW2],
                            start=False, stop=False))
                for k in range(KH):
                    mov = h0[:, k * CW + half * HW2:k * CW + (half + 1) * HW2]
                    for m in range(KH):
                        w = wg1_sb[:, k * H + m * 128:k * H + (m + 1) * 128]
                        cs = m * CW + half * HW2
                        thunks.append(
                            lambda w=w, mov=mov, cs=cs, k=k:
                            nc.tensor.matmul(
                                ps1[:, cs:cs + HW2], w, mov,
                                start=False, stop=(k == KH - 1),
                            ))
                return thunks

            # ---- one chain step (layer l) ----
            def step(l, c, r):
                ps = ps0_tiles[c] if l == 0 else ps1
                wh = wh0_sb if l == 0 else wh1_sb
                ht = h0_t if l == 0 else h1_t
                hsrc = ht[c] if r > 0 else ht[c - 1]
                rp = r - 1 if r > 0 else CH - 1
                if l == 1 and dbg_l1_mode == 1:
                    hsrc = h1_zero  # no recurrence: h1 = relu(b1 + proj)
                for k in range(KH):
                    ksrc = hsrc[:, k * CW + rp * NW:k * CW + (rp + 1) * NW]
                    for m in range(KH):
                        w = wh[:, k * H + m * 128:k * H + (m + 1) * 128]
                        nc.tensor.matmul(
                            mslice(ps, m, r), w, ksrc,
                            start=False, stop=(k == KH - 1),
                        )
                # relu psum -> h; ACT for L0, DVE for L1 (one sem per chain)
                h4 = ht[c][:].rearrange("p (k x) -> p k x", k=KH)
                p4 = ps[:].rearrange("p (m x) -> p m x", m=KH)
                if l == 0:
                    nc.scalar.activation(
                        h4[:, :, r * NW:(r + 1) * NW],
                        p4[:, :, r * NW:(r + 1) * NW],
                        mybir.ActivationFunctionType.Relu)
                else:
                    nc.vector.tensor_scalar_max(
                        h4[:, :, r * NW:(r + 1) * NW],
                        p4[:, :, r * NW:(r + 1) * NW], 0.0)

            # fc logits accumulate into one SBUF strip; sigmoid + DMA happen
            # ONCE at the end (per-iteration sigmoid forced an ACT
            # table-switch + DMA cadence worth ~20us/iter on HW).
            ylog = p_w.tile([1, nch * CW], f32, tag="ylog")

            def fc(c, j):
                ps = p_ps.tile([8, CW], f32, tag="fc", name=f"fc{c}")
                for k in range(KH):
                    nc.tensor.matmul(
                        ps[:], wfc_sb[:, k * 8:(k + 1) * 8],
                        h1_t[c][:, k * CW:(k + 1) * CW],
                        start=(k == 0), stop=(k == KH - 1),
                    )
                nc.vector.tensor_scalar_add(
                    ylog[0:1, c * CW:(c + 1) * CW], ps[0:1, :], 0.0)

            def final_sigmoid():
                ysb = p_y.tile([1, nch * CW], f32, tag="y", name="yfin")
                nc.scalar.activation(
                    ysb[:], ylog[:], mybir.ActivationFunctionType.Sigmoid,
                    bias=bfc_sb[0:1, 0:1],
                )
                nc.sync.dma_start(y_d[:], ysb[:])

            # ---- main loop ----
            for t in l0_thunks(0):
                t()
            for j in range(niter):
                c0 = j            # L0 chunk this iteration
                c1 = j - LAG1     # L1 chunk this iteration
                if c0 < nch:
                    h0_t[c0] = p_h0.tile([128, HC], bf, tag="h0",
                                         name=f"h0_{c0}")
                if 0 <= c1 < nch:
                    h1_t[c1] = p_h1.tile([128, HC], bf, tag="h1",
                                         name=f"h1_{c1}")
                # fillers: L0 proj for chunk j+1, spread across slots
                pend = l0_thunks(c0 + 1) if c0 + 1 < nch else []
                per = -(-len(pend) // CH) if pend else 0
                # refill schedule (half-chunks, maximal slack): memset at
                # slots 7/15, DVE-in-order right after the relu that frees
                # those cols; the PE refill matmuls land 5 slots later
                # (half0 of chunk c1+1 at slot 12, half1 at slot 4 of the
                # NEXT iteration) so their memset wait is long resolved.
                for r in range(CH):
                    if c0 < nch:
                        step(0, c0, r)
                    if 0 <= c1 < nch:
                        step(1, c1, r)
                    if r in (7, 15) and 0 <= c1 + 1 < nch:
                        l1_memset(r // 8)
                    rq = None
                    if r == 4 and 0 <= c1 < nch:
                        rq = (c1, 1)
                    elif r == 12 and 0 <= c1 + 1 < nch:
                        rq = (c1 + 1, 0)
                    if rq is not None:
                        for t in l1_subthunk(*rq):
                            t()
                    for t in pend[r * per:(r + 1) * per]:
                        t()
                cf = j - LAGF
                if not no_fc and 0 <= cf < nch:
                    fc(cf, j)
                if debug_dumps:
                    if c0 < nch:
                        nc.sync.dma_start(
                            dh_d[:, c0 * 2 * HC:c0 * 2 * HC + HC],
                            h0_t[c0][:])
                    if 0 <= c1 < nch:
                        nc.sync.dma_start(
                            dh_d[:, c1 * 2 * HC + HC:(c1 + 1) * 2 * HC],
                            h1_t[c1][:])
            if not no_fc:
                final_sigmoid()

    _split_multi_waits(nc)
    return nc


_cache = {}


def _get_nc(n_steps):
    if n_steps not in _cache:
        _cache[n_steps] = build_nc(n_steps)
    return _cache[n_steps]


def _prep_inputs(x, W_ih0, W_hh0, b_ih0, b_hh0, W_ih1, W_hh1, b_ih1, b_hh1,
                 W_fc, b_fc, n_steps=T):
    def pack_lhsT(w, kc):  # w [out(H), in(kc*128)] -> lhsT [128, kc*H]
        t = w.T.reshape(kc, 128, H).transpose(1, 0, 2)
        return np.ascontiguousarray(t.reshape(128, kc * H)).astype(F16)

    wfc = np.zeros((KH, 128, 8), np.float32)
    wfc[:, :, 0] = W_fc.reshape(KH, 128)
    wfc = np.ascontiguousarray(
        wfc.transpose(1, 0, 2).reshape(128, KH * 8)).astype(F16)

    b0 = (b_ih0 + b_hh0).reshape(KH, 128).astype(F16)
    b1 = (b_ih1 + b_hh1).reshape(KH, 128).astype(F16)
    ind = np.zeros((KH, HC), np.float32)
    for c in range(KH):
        ind[c, c * CW:(c + 1) * CW] = 1.0
    ind = ind.astype(F16)
    bfc = b_fc.reshape(1, 1).astype(np.float32)

    in_maps = []
    for c in range(NCORES):
        xs = x[c * NW:(c + 1) * NW, :n_steps]        # [16, t, I]
        # layout [kx][t][s]: value x[s, t, kx*128+i] at col
        # kx*(n_steps*NW) + t*NW + s
        xt = xs.reshape(NW, n_steps, KX, 128)        # [s][t][kx][i]
        xt = xt.transpose(3, 2, 1, 0)                # [i][kx][t][s]
        xt = np.ascontiguousarray(
            xt.reshape(128, KX * n_steps * NW)).astype(F16)
        in_maps.append({
            "xt": xt,
            "wg0": pack_lhsT(W_ih0, KX),
            "wg1": pack_lhsT(W_ih1, KH),
            "wh0": pack_lhsT(W_hh0, KH),
            "wh1": pack_lhsT(W_hh1, KH),
            "b0": b0,
            "b1": b1,
            "wfc": wfc,
            "bfc": bfc,
            "ind": ind,
        })
    return in_maps


def _postprocess(results, n_steps=T):
    nch = n_steps // CH
    out = np.zeros((B, n_steps, 1), np.float32)
    for p in range(NCORES):
        y = results[p]["y"].reshape(nch, CH, NW)     # [chunk][r][s]
        blk = y.transpose(2, 0, 1).reshape(NW, n_steps)  # [s][t]
        out[p * NW:(p + 1) * NW, :, 0] = blk
    return out


def kernel(x, W_ih0, W_hh0, b_ih0, b_hh0, W_ih1, W_hh1, b_ih1, b_hh1,
           W_fc, b_fc):
    args = [np.asarray(a, dtype=np.float32)
            for a in (x, W_ih0, W_hh0, b_ih0, b_hh0, W_ih1, W_hh1, b_ih1,
                      b_hh1, W_fc, b_fc)]
    nc = _get_nc(T)
    in_maps = _prep_inputs(*args)
    res = run_bass_kernel_spmd(nc, in_maps, core_ids=list(range(NCORES)))
    return _postprocess(res.results)
